# revision 33
# baseline (speedup 1.0000x reference)
"""AttnBlock (B=4, C=512, L=4096) distributed over 8 TRN2 NeuronCores.

Sharding: core i handles batch b = i//2, query half h = i%2 (rows
h*2048 .. h*2048+2048).  No collectives: each core receives the full
x[b] (rolled so its query half sits at columns 0:2048 -- attention is
permutation-invariant over key positions, so rolling K/V order only
changes fp accumulation order) and duplicates the LN + K/V projection
work with its pair core.

On-chip layout is fully transposed ([channel, seq] -- x's native
layout), so the kernel contains no transposes at all:
  h^T [c, l]   = LN(x) via partition-axis stats (gpsimd partition_all_reduce)
  Q^T,K^T[o,l] = WqT/WkT.T @ h^T          (matmul, contraction over c)
  V    [m, o]  = h^T.T @ WvT
  S^T  [m, l]  = K^T.T @ Q^T              ([key, query] layout, bf16)
  P^T  [m, l]  = exp(S^T / sqrt(C))       (no max subtraction: scores ~ N(0,1))
  O^T  [c, l]  = V.T @ P^T                (accumulated over m in PSUM, fp32r)
  softmax sums = partition_all_reduce over m of P^T, reciprocal, multiply
  out^T [o, l] = WpT.T @ O_norm^T; SELU; + x residual

Matmul dtypes: projections / PV / out-proj run at float32r (full PE
rate at N=512); QK^T runs at bf16.  The BIR verifier requires every
fp32r-matmul operand to be written by a compute op with an fp32r-typed
output (rounding on write), so weights are DMA-staged then ACT-copied,
and h / P^T / O_norm are written through fp32r-typed views.

ln_w / ln_b are folded into the projection weights on the host
(w_eff = w * ln_w, b_eff = w @ ln_b + b), so the device only computes
the plain (x - mu) * rsqrt(var + eps) normalization.
"""

import sys

for _p in ("/opt/trn_rl_repo", "/root/.axon_site/_ro/trn_rl_repo"):
    if _p not in sys.path:
        sys.path.insert(0, _p)

import numpy as np

import concourse.bass as bass  # noqa: F401  (re-exported for tests)
import concourse.bass_isa as bass_isa
import concourse.tile as tile
from concourse import bacc, mybir
from concourse.bass_utils import run_bass_kernel_spmd

B, C, L = 4, 512, 4096
HALF = L // 2
LN_EPS = 1e-5
NCHUNK = C // 128          # 4 channel chunks
LTILE = 512                # l-tile (free dim per matmul)
NLT_FULL = L // LTILE      # 8 l-tiles covering full sequence
NLT_Q = HALF // LTILE      # 4 l-tiles covering this core's queries
MCHUNK = L // 128          # 32 key chunks of 128
SELU_ALPHA = 1.6732632423543772848170429916717
SELU_SCALE = 1.0507009873554804934193349852946
LA = SELU_SCALE * SELU_ALPHA

F32 = mybir.dt.float32
F32R = mybir.dt.float32r
BF16 = mybir.dt.bfloat16
AF = mybir.ActivationFunctionType
ALU = mybir.AluOpType


def r(ap):
    return ap.bitcast(F32R)


def build_nc():
    nc = bacc.Bacc(
        "TRN2", target_bir_lowering=False, debug=False, num_devices=8
    )
    x_d = nc.dram_tensor("x", [C, L], F32, kind="ExternalInput").ap()
    wqT_d = nc.dram_tensor("wqT", [C, C], F32, kind="ExternalInput").ap()
    wkT_d = nc.dram_tensor("wkT", [C, C], F32, kind="ExternalInput").ap()
    wvT_d = nc.dram_tensor("wvT", [C, C], F32, kind="ExternalInput").ap()
    wpT_d = nc.dram_tensor("wpT", [C, C], F32, kind="ExternalInput").ap()
    bqk_d = nc.dram_tensor("bqk", [2, NCHUNK, 128], F32, kind="ExternalInput").ap()
    out_d = nc.dram_tensor("out", [C, HALF], F32, kind="ExternalOutput").ap()

    with tile.TileContext(nc) as tc:
        with (
            tc.tile_pool(name="pdram", bufs=1, space="DRAM") as pdram,
            tc.tile_pool(name="pw", bufs=1) as pw,
            tc.tile_pool(name="pkv", bufs=1) as pkv,
            tc.tile_pool(name="px", bufs=2) as px,
            tc.tile_pool(name="ph", bufs=2) as ph,
            tc.tile_pool(name="pstat", bufs=6) as pstat,
            tc.tile_pool(name="pq", bufs=2) as pq,
            tc.tile_pool(name="pp", bufs=5) as pp,
            tc.tile_pool(name="pon", bufs=2) as pon,
            tc.tile_pool(name="psS", bufs=2, space="PSUM") as psS,
            tc.tile_pool(name="psPV", bufs=1, space="PSUM") as psPV,
        ):
            qspill = pdram.tile([C, HALF], BF16, tag="qspill")

            # prefetch the first x l-tile before weight staging so the LN
            # chain (the startup critical path) starts at t=0
            X0 = px.tile([128, NCHUNK, LTILE], F32, tag="X", name="X0")
            for ci in range(NCHUNK):
                nc.sync.dma_start(
                    out=X0[:, ci, :], in_=x_d[ci * 128:(ci + 1) * 128, 0:LTILE]
                )

            # ---- resident weights: DMA-stage then ACT-copy to bf16 ----
            wq_s = pw.tile([128, NCHUNK, C], BF16, tag="wq")
            wk_s = pw.tile([128, NCHUNK, C], BF16, tag="wk")
            wv_s = pw.tile([128, NCHUNK, C], BF16, tag="wv")
            for ci in range(NCHUNK):
                for w_d, w_s in ((wkT_d, wk_s), (wvT_d, wv_s), (wqT_d, wq_s)):
                    stg = pp.tile([128, C], F32, tag="pp", name="stg")
                    nc.sync.dma_start(
                        out=stg[:], in_=w_d[ci * 128:(ci + 1) * 128, :]
                    )
                    nc.scalar.copy(w_s[:, ci, :], stg[:])
            bqk_s = pw.tile([128, 2, NCHUNK], F32, tag="bqk")
            for which in range(2):
                for oc in range(NCHUNK):
                    nc.sync.dma_start(
                        out=bqk_s[:, which, oc:oc + 1], in_=bqk_d[which, oc, :]
                    )
            eps_t = pw.tile([128, 1], F32, tag="eps")
            nc.vector.memset(eps_t[:], LN_EPS)

            # ---- K/V: local staging + rank-ordered gathered copy ----
            # layout [128, slot, ko, 512]: ko 0..3 = K^T o-chunks, 4..7 = V
            # m-chunks; kv_gath slots 0..3 = rank0 l-tiles, 4..7 = rank1
            kv_gath = pkv.tile([128, 2 * NLT_Q, 8, LTILE], BF16, tag="kvg")
            Hs = []

            # spin the PE on zeros during the first LN chain so the HAM clock
            # gate is fully open (2.4 GHz) when real matmuls arrive
            warm_w = pw.tile([128, 128], BF16, tag="warmw")
            nc.vector.memset(warm_w[:], 0.0)
            warm_z = pw.tile([128, LTILE], BF16, tag="warmz")
            nc.vector.memset(warm_z[:], 0.0)
            warm_ps = psPV.tile([128, NCHUNK, LTILE], F32, tag="pvall", name="warm_ps")
            for wi in range(70):
                nc.tensor.matmul(
                    warm_ps[:, wi % NCHUNK, :],
                    warm_w[:],
                    warm_z[:],
                    start=True,
                    stop=True,
                )

            # ====== Phase 1: LN + Q/K/V projections (local query half only;
            # K/V for the other half arrive via pair AllGather) ======
            for lt in range(NLT_Q):
                ls = lt * LTILE
                if lt == 0:
                    X = X0
                else:
                    X = px.tile([128, NCHUNK, LTILE], F32, tag="X")
                    for ci in range(NCHUNK):
                        nc.sync.dma_start(
                            out=X[:, ci, :],
                            in_=x_d[ci * 128:(ci + 1) * 128, ls:ls + LTILE],
                        )
                # partial sums over the 4 channel chunks
                sx = pstat.tile([128, LTILE], F32, tag="st")
                sxx = pstat.tile([128, LTILE], F32, tag="st")
                t0 = pstat.tile([128, LTILE], F32, tag="st")
                nc.vector.tensor_tensor(sx, X[:, 0, :], X[:, 1, :], ALU.add)
                nc.vector.tensor_tensor(t0, X[:, 2, :], X[:, 3, :], ALU.add)
                nc.vector.tensor_tensor(sx, sx, t0, ALU.add)
                sq0 = pstat.tile([128, LTILE], F32, tag="st")
                sq1 = pstat.tile([128, LTILE], F32, tag="st")
                nc.scalar.square(sxx, X[:, 0, :])
                nc.scalar.square(sq0, X[:, 1, :])
                nc.vector.tensor_tensor(sxx, sxx, sq0, ALU.add)
                nc.scalar.square(sq1, X[:, 2, :])
                nc.scalar.square(sq0, X[:, 3, :])
                nc.vector.tensor_tensor(sq1, sq1, sq0, ALU.add)
                nc.vector.tensor_tensor(sxx, sxx, sq1, ALU.add)
                # partition all-reduce -> every partition holds the full sums
                bsx = pstat.tile([128, LTILE], F32, tag="st")
                bsxx = pstat.tile([128, LTILE], F32, tag="st")
                nc.gpsimd.partition_all_reduce(
                    bsx[:], sx[:], 128, bass_isa.ReduceOp.add
                )
                nc.gpsimd.partition_all_reduce(
                    bsxx[:], sxx[:], 128, bass_isa.ReduceOp.add
                )
                # rr = rsqrt(E[x^2] - mu^2 + eps); mu = E[x]
                mu = pstat.tile([128, LTILE], F32, tag="st")
                nc.vector.tensor_scalar(mu, bsx, 1.0 / C, None, op0=ALU.mult)
                var = pstat.tile([128, LTILE], F32, tag="st")
                nc.vector.tensor_scalar(var, bsxx, 1.0 / C, None, op0=ALU.mult)
                mu2 = pstat.tile([128, LTILE], F32, tag="st")
                nc.vector.tensor_tensor(mu2, mu, mu, ALU.mult)
                nc.vector.tensor_tensor(var, var, mu2, ALU.subtract)
                sd = pstat.tile([128, LTILE], F32, tag="st")
                nc.scalar.activation(sd, var, AF.Sqrt, bias=eps_t[:])
                rr = pstat.tile([128, LTILE], F32, tag="st")
                nc.vector.reciprocal_approx_fast(out=rr[:], in_=sd[:])
                # X <- X - mu (in place);  h <- X * rr  (bf16 write)
                H = ph.tile([128, NCHUNK, LTILE], BF16, tag="H")
                Hs.append(H)
                for ci in range(NCHUNK):
                    nc.vector.tensor_tensor(
                        X[:, ci, :], X[:, ci, :], mu, ALU.subtract
                    )
                    nc.vector.tensor_tensor(
                        H[:, ci, :], X[:, ci, :], rr, ALU.mult
                    )
                # K^T projection into the local K/V staging block
                kv_loc = pkv.tile([128, 8, LTILE], BF16, tag="kvl", bufs=2)
                for oc in range(0, NCHUNK, 2):
                    ps = psS.tile([128, 2, LTILE], F32, tag="ps")
                    for half in range(2):
                        for ci in range(NCHUNK):
                            nc.tensor.matmul(
                                ps[:, half, :],
                                wk_s[:, ci, (oc + half) * 128:(oc + half + 1) * 128],
                                H[:, ci, :],
                                start=(ci == 0),
                                stop=(ci == NCHUNK - 1),
                            )
                    for half in range(2):
                        nc.scalar.activation(
                            kv_loc[:, oc + half, :], ps[:, half, :],
                            AF.Identity, bias=bqk_s[:, 1, oc + half:oc + half + 1],
                        )
                # V projection into the local K/V block
                for mc in range(0, NCHUNK, 2):
                    ps = psS.tile([128, 2, LTILE], F32, tag="ps")
                    for half in range(2):
                        for ci in range(NCHUNK):
                            nc.tensor.matmul(
                                ps[:, half, :],
                                H[:, ci, (mc + half) * 128:(mc + half + 1) * 128],
                                wv_s[:, ci, :],
                                start=(ci == 0),
                                stop=(ci == NCHUNK - 1),
                            )
                    nc.scalar.copy(kv_loc[:, 4 + mc:4 + mc + 2, :], ps[:])
                # pair AllGather of this l-tile's K/V block via DRAM bounce
                kv_in = pdram.tile(
                    [128, 8, LTILE], BF16, tag="kvi", bufs=2, name="kv_in"
                )
                kv_out = pdram.tile(
                    [2, 128, 8, LTILE], BF16, tag="kvo", bufs=2, name="kv_out"
                )
                nc.sync.dma_start(out=kv_in[:], in_=kv_loc[:])
                nc.gpsimd.collective_compute(
                    "AllGather",
                    ALU.bypass,
                    replica_groups=[[0, 1], [2, 3], [4, 5], [6, 7]],
                    ins=[kv_in.opt()],
                    outs=[kv_out.opt()],
                )
                for rk in range(2):
                    nc.sync.dma_start(
                        out=kv_gath[:, rk * NLT_Q + lt, :, :], in_=kv_out[rk]
                    )
                # Q^T projection on the (phase-1-idle) psPV banks
                qps = psPV.tile(
                    [128, NCHUNK, LTILE], F32, tag="pvall", name="qps"
                )
                for oc in range(NCHUNK):
                    for ci in range(NCHUNK):
                        nc.tensor.matmul(
                            qps[:, oc, :],
                            wq_s[:, ci, oc * 128:(oc + 1) * 128],
                            Hs[lt][:, ci, :],
                            start=(ci == 0),
                            stop=(ci == NCHUNK - 1),
                        )
                for oc in range(NCHUNK):
                    qt = pp.tile([128, LTILE], BF16, tag="ppb", name="qt")
                    nc.scalar.activation(
                        qt, qps[:, oc, :], AF.Identity,
                        bias=bqk_s[:, 0, oc:oc + 1],
                    )
                    nc.sync.dma_start(
                        out=qspill[oc * 128:(oc + 1) * 128, ls:ls + LTILE],
                        in_=qt[:],
                    )

            # wp loaded after phase 1
            wp_s = pw.tile([128, NCHUNK, C], BF16, tag="wp", name="wp_s")
            for ci in range(NCHUNK):
                stg = pp.tile([128, C], F32, tag="pp", name="stgp")
                nc.sync.dma_start(
                    out=stg[:], in_=wpT_d[ci * 128:(ci + 1) * 128, :]
                )
                nc.scalar.copy(wp_s[:, ci, :], stg[:])

            # keep the PE clock warm across the phase transition
            warm_ps2 = psPV.tile(
                [128, NCHUNK, LTILE], F32, tag="pvall", name="warm_ps2"
            )
            for wi in range(20):
                nc.tensor.matmul(
                    warm_ps2[:, wi % NCHUNK, :],
                    warm_w[:],
                    warm_z[:],
                    start=True,
                    stop=True,
                )

            # ============ Phase 2+3: attention + out-proj per l-tile =======
            inv_sqrt_c = 1.0 / float(np.sqrt(C))
            NPAIR = MCHUNK // 2
            for lt in range(NLT_Q):
                ls = lt * LTILE
                qT = pq.tile([128, NCHUNK, LTILE], BF16, tag="qT")
                for oc in range(NCHUNK):
                    nc.sync.dma_start(
                        out=qT[:, oc, :],
                        in_=qspill[oc * 128:(oc + 1) * 128, ls:ls + LTILE],
                    )
                xrs = []
                for ohc in range(NCHUNK):
                    xr = pp.tile([128, LTILE], F32, tag="xr", name="xr", bufs=4)
                    nc.sync.dma_start(
                        out=xr[:],
                        in_=x_d[ohc * 128:(ohc + 1) * 128, ls:ls + LTILE],
                    )
                    xrs.append(xr)
                # two double-width accumulators; fold + all-reduce in halves so
                # the first gpsimd pass hides under the second half of the loop
                sumsA = pstat.tile([128, 2 * LTILE], F32, tag="sw", name="sumsA", bufs=3)
                sumsB = pstat.tile([128, 2 * LTILE], F32, tag="sw", name="sumsB", bufs=3)
                nc.vector.memset(sumsA[:], 0.0)
                nc.vector.memset(sumsB[:], 0.0)
                bsAi = pstat.tile([128, LTILE], F32, tag="st", name="bsAi")
                bsBi = pstat.tile([128, LTILE], F32, tag="st", name="bsBi")
                bsA = pstat.tile([128, LTILE], F32, tag="st", name="bsA")
                bsB = pstat.tile([128, LTILE], F32, tag="st", name="bsB")
                pv = psPV.tile(
                    [128, NCHUNK, LTILE], F32, tag="pvall", name="pv"
                )
                # consume gathered K/V in collective-arrival order:
                # slot rk*4+lt, ordered by lt (the collective issue order)
                SLOTS = [0, 4, 1, 5, 2, 6, 3, 7]
                for jj in range(NPAIR):
                    sT = psS.tile([128, 2, LTILE], F32, tag="ps")
                    for half in range(2):
                        j = 2 * jj + half
                        slt, mc = SLOTS[j // NCHUNK], j % NCHUNK
                        for oc in range(NCHUNK):
                            nc.tensor.matmul(
                                sT[:, half, :],
                                kv_gath[:, slt, oc, mc * 128:(mc + 1) * 128],
                                qT[:, oc, :],
                                start=(oc == 0),
                                stop=(oc == NCHUNK - 1),
                            )
                    pT = pp.tile([128, 2, LTILE], BF16, tag="ppb", name="pT")
                    nc.scalar.activation(
                        pT[:], sT[:], AF.Exp, scale=inv_sqrt_c
                    )
                    acc = sumsA if jj < NPAIR // 2 else sumsB
                    nc.vector.tensor_tensor(
                        acc.rearrange("p (a b) -> p a b", a=2), acc.rearrange("p (a b) -> p a b", a=2), pT[:], ALU.add
                    )
                    for half in range(2):
                        j = 2 * jj + half
                        slt, mc = SLOTS[j // NCHUNK], j % NCHUNK
                        for cc in range(NCHUNK):
                            nc.tensor.matmul(
                                pv[:, cc, :],
                                kv_gath[:, slt, 4 + mc, cc * 128:(cc + 1) * 128],
                                pT[:, half, :],
                                start=(j == 0),
                                stop=(j == MCHUNK - 1),
                            )
                    if jj == NPAIR // 2 - 1:
                        nc.vector.tensor_tensor(
                            bsAi, sumsA[:, 0:LTILE], sumsA[:, LTILE:2 * LTILE],
                            ALU.add,
                        )
                        nc.gpsimd.partition_all_reduce(
                            bsA[:], bsAi[:], 128, bass_isa.ReduceOp.add
                        )
                nc.vector.tensor_tensor(
                    bsBi, sumsB[:, 0:LTILE], sumsB[:, LTILE:2 * LTILE], ALU.add
                )
                nc.gpsimd.partition_all_reduce(
                    bsB[:], bsBi[:], 128, bass_isa.ReduceOp.add
                )
                bs = pstat.tile([128, LTILE], F32, tag="st", name="bs")
                nc.vector.tensor_tensor(bs, bsA, bsB, ALU.add)
                rs = pstat.tile([128, LTILE], F32, tag="st", name="rs")
                nc.vector.reciprocal_approx_fast(out=rs[:], in_=bs[:])
                # unnormalized O^T -> bf16 (normalization deferred past out-proj)
                on = pon.tile([128, NCHUNK, LTILE], BF16, tag="on", name="on", bufs=2)
                nc.vector.tensor_copy(on[:], pv[:])
                # out-projection (reuses the PV PSUM banks); then normalize,
                # SELU, residual
                po = psPV.tile(
                    [128, NCHUNK, LTILE], F32, tag="pvall", name="po"
                )
                for oc in range(NCHUNK):
                    for cc in range(NCHUNK):
                        nc.tensor.matmul(
                            po[:, oc, :],
                            wp_s[:, cc, oc * 128:(oc + 1) * 128],
                            on[:, cc, :],
                            start=(cc == 0),
                            stop=(cc == NCHUNK - 1),
                        )
                if True:
                    for ohc in range(NCHUNK):
                        z = pp.tile([128, LTILE], F32, tag="pp", name="z")
                        nc.vector.tensor_tensor(z, po[:, ohc, :], rs, ALU.mult)
                        e = pp.tile([128, LTILE], F32, tag="pp", name="e")
                        nc.scalar.activation(e, z[:], AF.Exp)
                        # e <- LA * min(e, 1)
                        nc.vector.tensor_scalar(
                            e, e, 1.0, LA, op0=ALU.min, op1=ALU.mult
                        )
                        b2 = pp.tile([128, LTILE], F32, tag="pp", name="b2")
                        # b2 <- SCALE * max(z, 0)
                        nc.vector.tensor_scalar(
                            b2, z[:], 0.0, SELU_SCALE, op0=ALU.max, op1=ALU.mult
                        )
                        nc.vector.tensor_tensor(e, e, b2, ALU.add)
                        nc.vector.tensor_scalar(e, e, LA, None, op0=ALU.subtract)
                        ot = pp.tile([128, LTILE], F32, tag="ot", name="ot", bufs=3)
                        nc.vector.tensor_tensor(ot, e, xrs[ohc], ALU.add)
                        nc.sync.dma_start(
                            out=out_d[ohc * 128:(ohc + 1) * 128, ls:ls + LTILE],
                            in_=ot[:],
                        )

    nc.compile()
    return nc


_CACHED_NC = None


def _get_nc():
    global _CACHED_NC
    if _CACHED_NC is None:
        _CACHED_NC = build_nc()
    return _CACHED_NC


def make_in_maps(x, ln_w, ln_b, wq, bq, wk, bk, wv, bv, wp, bp):
    x = np.ascontiguousarray(np.asarray(x, np.float32))
    ln_w = np.asarray(ln_w, np.float32)
    ln_b = np.asarray(ln_b, np.float32)

    def eff(w, b):
        w = np.asarray(w, np.float32)
        b = np.asarray(b, np.float32)
        w_eff = w * ln_w[None, :]
        b_eff = w @ ln_b + b
        return np.ascontiguousarray(w_eff.T), b_eff

    wqT, bq_e = eff(wq, bq)
    wkT, bk_e = eff(wk, bk)
    wvT, bv_e = eff(wv, bv)
    wpT = np.ascontiguousarray(np.asarray(wp, np.float32).T)
    assert not np.any(bv_e), "nonzero v bias not supported by the graph"
    assert not np.any(np.asarray(bp, np.float32)), "nonzero p bias not supported"
    bqk = np.ascontiguousarray(np.stack([bq_e, bk_e]).reshape(2, NCHUNK, 128))

    in_maps = []
    for i in range(8):
        b, h = i // 2, i % 2
        if h == 0:
            xs = x[b]
        else:
            xs = np.ascontiguousarray(
                np.concatenate([x[b][:, HALF:], x[b][:, :HALF]], axis=1)
            )
        in_maps.append(
            {
                "x": xs,
                "wqT": wqT,
                "wkT": wkT,
                "wvT": wvT,
                "wpT": wpT,
                "bqk": bqk,
            }
        )
    return in_maps


def assemble(results):
    out = np.empty((B, C, L), np.float32)
    for i in range(8):
        b, h = i // 2, i % 2
        out[b][:, h * HALF:(h + 1) * HALF] = results[i]["out"]
    return out


def kernel(**inputs):
    nc = _get_nc()
    in_maps = make_in_maps(**inputs)
    res = run_bass_kernel_spmd(nc, in_maps, core_ids=list(range(8)))
    return assemble(res.results)


if __name__ == "__main__":
    build_nc()
    print("built + compiled OK")


# revision 34
# speedup vs baseline: 1.1014x; 1.1014x over previous
"""AttnBlock (B=4, C=512, L=4096) distributed over 8 TRN2 NeuronCores.

Sharding: core i handles batch b = i//2, query half h = i%2 (rows
h*2048 .. h*2048+2048).  No collectives: each core receives the full
x[b] (rolled so its query half sits at columns 0:2048 -- attention is
permutation-invariant over key positions, so rolling K/V order only
changes fp accumulation order) and duplicates the LN + K/V projection
work with its pair core.

On-chip layout is fully transposed ([channel, seq] -- x's native
layout), so the kernel contains no transposes at all:
  h^T [c, l]   = LN(x) via partition-axis stats (gpsimd partition_all_reduce)
  Q^T,K^T[o,l] = WqT/WkT.T @ h^T          (matmul, contraction over c)
  V    [m, o]  = h^T.T @ WvT
  S^T  [m, l]  = K^T.T @ Q^T              ([key, query] layout, bf16)
  P^T  [m, l]  = exp(S^T / sqrt(C))       (no max subtraction: scores ~ N(0,1))
  O^T  [c, l]  = V.T @ P^T                (accumulated over m in PSUM, fp32r)
  softmax sums = partition_all_reduce over m of P^T, reciprocal, multiply
  out^T [o, l] = WpT.T @ O_norm^T; SELU; + x residual

Matmul dtypes: projections / PV / out-proj run at float32r (full PE
rate at N=512); QK^T runs at bf16.  The BIR verifier requires every
fp32r-matmul operand to be written by a compute op with an fp32r-typed
output (rounding on write), so weights are DMA-staged then ACT-copied,
and h / P^T / O_norm are written through fp32r-typed views.

ln_w / ln_b are folded into the projection weights on the host
(w_eff = w * ln_w, b_eff = w @ ln_b + b), so the device only computes
the plain (x - mu) * rsqrt(var + eps) normalization.
"""

import sys

for _p in ("/opt/trn_rl_repo", "/root/.axon_site/_ro/trn_rl_repo"):
    if _p not in sys.path:
        sys.path.insert(0, _p)

import numpy as np

import concourse.bass as bass  # noqa: F401  (re-exported for tests)
import concourse.bass_isa as bass_isa
import concourse.tile as tile
from concourse import bacc, mybir
from concourse.bass_utils import run_bass_kernel_spmd

B, C, L = 4, 512, 4096
HALF = L // 2
LN_EPS = 1e-5
NCHUNK = C // 128          # 4 channel chunks
LTILE = 512                # l-tile (free dim per matmul)
NLT_FULL = L // LTILE      # 8 l-tiles covering full sequence
NLT_Q = HALF // LTILE      # 4 l-tiles covering this core's queries
MCHUNK = L // 128          # 32 key chunks of 128
SELU_ALPHA = 1.6732632423543772848170429916717
SELU_SCALE = 1.0507009873554804934193349852946
LA = SELU_SCALE * SELU_ALPHA

F32 = mybir.dt.float32
F32R = mybir.dt.float32r
BF16 = mybir.dt.bfloat16
AF = mybir.ActivationFunctionType
ALU = mybir.AluOpType


def r(ap):
    return ap.bitcast(F32R)


def build_nc():
    nc = bacc.Bacc(
        "TRN2", target_bir_lowering=False, debug=False, num_devices=8
    )
    x_d = nc.dram_tensor("x", [C, L], F32, kind="ExternalInput").ap()
    wqT_d = nc.dram_tensor("wqT", [C, C], F32, kind="ExternalInput").ap()
    wkT_d = nc.dram_tensor("wkT", [C, C], F32, kind="ExternalInput").ap()
    wvT_d = nc.dram_tensor("wvT", [C, C], F32, kind="ExternalInput").ap()
    wpT_d = nc.dram_tensor("wpT", [C, C], F32, kind="ExternalInput").ap()
    bqk_d = nc.dram_tensor("bqk", [2, NCHUNK, 128], F32, kind="ExternalInput").ap()
    out_d = nc.dram_tensor("out", [C, HALF], F32, kind="ExternalOutput").ap()

    with tile.TileContext(nc) as tc:
        with (
            tc.tile_pool(name="pdram", bufs=1, space="DRAM") as pdram,
            tc.tile_pool(name="pw", bufs=1) as pw,
            tc.tile_pool(name="pkv", bufs=1) as pkv,
            tc.tile_pool(name="px", bufs=2) as px,
            tc.tile_pool(name="ph", bufs=4) as ph,
            tc.tile_pool(name="pstat", bufs=6) as pstat,
            tc.tile_pool(name="pq", bufs=2) as pq,
            tc.tile_pool(name="pp", bufs=5) as pp,
            tc.tile_pool(name="pon", bufs=2) as pon,
            tc.tile_pool(name="psS", bufs=2, space="PSUM") as psS,
            tc.tile_pool(name="psPV", bufs=1, space="PSUM") as psPV,
        ):
            qspill = pdram.tile([C, HALF], BF16, tag="qspill")

            # prefetch the first x l-tile before weight staging so the LN
            # chain (the startup critical path) starts at t=0
            X0 = px.tile([128, NCHUNK, LTILE], F32, tag="X", name="X0")
            for ci in range(NCHUNK):
                nc.sync.dma_start(
                    out=X0[:, ci, :], in_=x_d[ci * 128:(ci + 1) * 128, 0:LTILE]
                )

            # tiny AllGather up front: absorbs the collective engine's
            # cold-start latency before the real K/V gathers need it
            dmy_in = pdram.tile([1, 1], F32, tag="dmyi")
            dmy_out = pdram.tile([2, 1], F32, tag="dmyo")
            dmy_s = pw.tile([1, 1], F32, tag="dmys")
            nc.vector.memset(dmy_s[:], 0.0)
            nc.sync.dma_start(out=dmy_in[:], in_=dmy_s[:])
            nc.gpsimd.collective_compute(
                "AllGather",
                ALU.bypass,
                replica_groups=[[0, 1], [2, 3], [4, 5], [6, 7]],
                ins=[dmy_in.opt()],
                outs=[dmy_out.opt()],
            )

            # ---- resident weights: DMA-stage then ACT-copy to bf16 ----
            wq_s = pw.tile([128, NCHUNK, C], BF16, tag="wq")
            wk_s = pw.tile([128, NCHUNK, C], BF16, tag="wk")
            wv_s = pw.tile([128, NCHUNK, C], BF16, tag="wv")
            for ci in range(NCHUNK):
                for w_d, w_s in ((wqT_d, wq_s), (wkT_d, wk_s), (wvT_d, wv_s)):
                    stg = pp.tile([128, C], F32, tag="pp", name="stg")
                    nc.sync.dma_start(
                        out=stg[:], in_=w_d[ci * 128:(ci + 1) * 128, :]
                    )
                    nc.scalar.copy(w_s[:, ci, :], stg[:])
            bqk_s = pw.tile([128, 2, NCHUNK], F32, tag="bqk")
            for which in range(2):
                for oc in range(NCHUNK):
                    nc.sync.dma_start(
                        out=bqk_s[:, which, oc:oc + 1], in_=bqk_d[which, oc, :]
                    )
            eps_t = pw.tile([128, 1], F32, tag="eps")
            nc.vector.memset(eps_t[:], LN_EPS)

            # ---- K/V: local staging + rank-ordered gathered copy ----
            # layout [128, slot, ko, 512]: ko 0..3 = K^T o-chunks, 4..7 = V
            # m-chunks; kv_gath slots 0..3 = rank0 l-tiles, 4..7 = rank1
            kv_gath = pkv.tile([128, 2 * NLT_Q, 8, LTILE], BF16, tag="kvg")
            Hs = []

            # spin the PE on zeros during the first LN chain so the HAM clock
            # gate is fully open (2.4 GHz) when real matmuls arrive
            warm_w = pw.tile([128, 128], BF16, tag="warmw")
            nc.vector.memset(warm_w[:], 0.0)
            warm_z = pw.tile([128, LTILE], BF16, tag="warmz")
            nc.vector.memset(warm_z[:], 0.0)
            warm_ps = psPV.tile([128, NCHUNK, LTILE], F32, tag="pvall", name="warm_ps")
            for wi in range(40):
                nc.tensor.matmul(
                    warm_ps[:, wi % NCHUNK, :],
                    warm_w[:],
                    warm_z[:],
                    start=True,
                    stop=True,
                )

            # ====== Phase 1: LN + Q/K/V projections (local query half only;
            # K/V for the other half arrive via pair AllGather) ======
            for lt in range(NLT_Q):
                ls = lt * LTILE
                if lt == 0:
                    X = X0
                else:
                    X = px.tile([128, NCHUNK, LTILE], F32, tag="X")
                    for ci in range(NCHUNK):
                        nc.sync.dma_start(
                            out=X[:, ci, :],
                            in_=x_d[ci * 128:(ci + 1) * 128, ls:ls + LTILE],
                        )
                # partial sums over the 4 channel chunks
                sx = pstat.tile([128, LTILE], F32, tag="st")
                sxx = pstat.tile([128, LTILE], F32, tag="st")
                t0 = pstat.tile([128, LTILE], F32, tag="st")
                nc.vector.tensor_tensor(sx, X[:, 0, :], X[:, 1, :], ALU.add)
                nc.vector.tensor_tensor(t0, X[:, 2, :], X[:, 3, :], ALU.add)
                nc.vector.tensor_tensor(sx, sx, t0, ALU.add)
                sq0 = pstat.tile([128, LTILE], F32, tag="st")
                sq1 = pstat.tile([128, LTILE], F32, tag="st")
                nc.scalar.square(sxx, X[:, 0, :])
                nc.scalar.square(sq0, X[:, 1, :])
                nc.vector.tensor_tensor(sxx, sxx, sq0, ALU.add)
                nc.scalar.square(sq1, X[:, 2, :])
                nc.scalar.square(sq0, X[:, 3, :])
                nc.vector.tensor_tensor(sq1, sq1, sq0, ALU.add)
                nc.vector.tensor_tensor(sxx, sxx, sq1, ALU.add)
                # partition all-reduce -> every partition holds the full sums
                bsx = pstat.tile([128, LTILE], F32, tag="st")
                bsxx = pstat.tile([128, LTILE], F32, tag="st")
                nc.gpsimd.partition_all_reduce(
                    bsx[:], sx[:], 128, bass_isa.ReduceOp.add
                )
                nc.gpsimd.partition_all_reduce(
                    bsxx[:], sxx[:], 128, bass_isa.ReduceOp.add
                )
                # rr = rsqrt(E[x^2] - mu^2 + eps); mu = E[x]
                mu = pstat.tile([128, LTILE], F32, tag="st")
                nc.vector.tensor_scalar(mu, bsx, 1.0 / C, None, op0=ALU.mult)
                var = pstat.tile([128, LTILE], F32, tag="st")
                nc.vector.tensor_scalar(var, bsxx, 1.0 / C, None, op0=ALU.mult)
                mu2 = pstat.tile([128, LTILE], F32, tag="st")
                nc.vector.tensor_tensor(mu2, mu, mu, ALU.mult)
                nc.vector.tensor_tensor(var, var, mu2, ALU.subtract)
                sd = pstat.tile([128, LTILE], F32, tag="st")
                nc.scalar.activation(sd, var, AF.Sqrt, bias=eps_t[:])
                rr = pstat.tile([128, LTILE], F32, tag="st")
                nc.vector.reciprocal_approx_fast(out=rr[:], in_=sd[:])
                # X <- X - mu (in place);  h <- X * rr  (bf16 write)
                H = ph.tile([128, NCHUNK, LTILE], BF16, tag="H")
                Hs.append(H)
                for ci in range(NCHUNK):
                    nc.vector.tensor_tensor(
                        X[:, ci, :], X[:, ci, :], mu, ALU.subtract
                    )
                    nc.vector.tensor_tensor(
                        H[:, ci, :], X[:, ci, :], rr, ALU.mult
                    )
                # K^T projection into the local K/V staging block
                kv_loc = pkv.tile([128, 8, LTILE], BF16, tag="kvl", bufs=2)
                for oc in range(0, NCHUNK, 2):
                    ps = psS.tile([128, 2, LTILE], F32, tag="ps")
                    for half in range(2):
                        for ci in range(NCHUNK):
                            nc.tensor.matmul(
                                ps[:, half, :],
                                wk_s[:, ci, (oc + half) * 128:(oc + half + 1) * 128],
                                H[:, ci, :],
                                start=(ci == 0),
                                stop=(ci == NCHUNK - 1),
                            )
                    for half in range(2):
                        nc.scalar.activation(
                            kv_loc[:, oc + half, :], ps[:, half, :],
                            AF.Identity, bias=bqk_s[:, 1, oc + half:oc + half + 1],
                        )
                # V projection into the local K/V block
                for mc in range(0, NCHUNK, 2):
                    ps = psS.tile([128, 2, LTILE], F32, tag="ps")
                    for half in range(2):
                        for ci in range(NCHUNK):
                            nc.tensor.matmul(
                                ps[:, half, :],
                                H[:, ci, (mc + half) * 128:(mc + half + 1) * 128],
                                wv_s[:, ci, :],
                                start=(ci == 0),
                                stop=(ci == NCHUNK - 1),
                            )
                    nc.scalar.copy(kv_loc[:, 4 + mc:4 + mc + 2, :], ps[:])
                # pair AllGather of this l-tile's K/V block via DRAM bounce
                kv_in = pdram.tile(
                    [128, 8, LTILE], BF16, tag="kvi", bufs=2, name="kv_in"
                )
                kv_out = pdram.tile(
                    [2, 128, 8, LTILE], BF16, tag="kvo", bufs=2, name="kv_out"
                )
                nc.sync.dma_start(out=kv_in[:], in_=kv_loc[:])
                nc.gpsimd.collective_compute(
                    "AllGather",
                    ALU.bypass,
                    replica_groups=[[0, 1], [2, 3], [4, 5], [6, 7]],
                    ins=[kv_in.opt()],
                    outs=[kv_out.opt()],
                )
                for rk in range(2):
                    nc.sync.dma_start(
                        out=kv_gath[:, rk * NLT_Q + lt, :, :], in_=kv_out[rk]
                    )
                # Q^T projection on the (phase-1-idle) psPV banks
                qps = psPV.tile(
                    [128, NCHUNK, LTILE], F32, tag="pvall", name="qps"
                )
                for oc in range(NCHUNK):
                    for ci in range(NCHUNK):
                        nc.tensor.matmul(
                            qps[:, oc, :],
                            wq_s[:, ci, oc * 128:(oc + 1) * 128],
                            Hs[lt][:, ci, :],
                            start=(ci == 0),
                            stop=(ci == NCHUNK - 1),
                        )
                for oc in range(NCHUNK):
                    qt = pp.tile([128, LTILE], BF16, tag="ppb", name="qt")
                    nc.scalar.activation(
                        qt, qps[:, oc, :], AF.Identity,
                        bias=bqk_s[:, 0, oc:oc + 1],
                    )
                    nc.sync.dma_start(
                        out=qspill[oc * 128:(oc + 1) * 128, ls:ls + LTILE],
                        in_=qt[:],
                    )

            # wp loaded after phase 1
            wp_s = pw.tile([128, NCHUNK, C], BF16, tag="wp", name="wp_s")
            for ci in range(NCHUNK):
                stg = pp.tile([128, C], F32, tag="pp", name="stgp")
                nc.sync.dma_start(
                    out=stg[:], in_=wpT_d[ci * 128:(ci + 1) * 128, :]
                )
                nc.scalar.copy(wp_s[:, ci, :], stg[:])

            # ============ Phase 2+3: attention + out-proj per l-tile =======
            inv_sqrt_c = 1.0 / float(np.sqrt(C))
            NPAIR = MCHUNK // 2
            for lt in range(NLT_Q):
                ls = lt * LTILE
                qT = pq.tile([128, NCHUNK, LTILE], BF16, tag="qT")
                for oc in range(NCHUNK):
                    nc.sync.dma_start(
                        out=qT[:, oc, :],
                        in_=qspill[oc * 128:(oc + 1) * 128, ls:ls + LTILE],
                    )
                xrs = []
                for ohc in range(NCHUNK):
                    xr = pp.tile([128, LTILE], F32, tag="xr", name="xr", bufs=4)
                    nc.sync.dma_start(
                        out=xr[:],
                        in_=x_d[ohc * 128:(ohc + 1) * 128, ls:ls + LTILE],
                    )
                    xrs.append(xr)
                # two double-width accumulators; fold + all-reduce in halves so
                # the first gpsimd pass hides under the second half of the loop
                sumsA = pstat.tile([128, 2 * LTILE], F32, tag="sw", name="sumsA", bufs=3)
                sumsB = pstat.tile([128, 2 * LTILE], F32, tag="sw", name="sumsB", bufs=3)
                nc.vector.memset(sumsA[:], 0.0)
                nc.vector.memset(sumsB[:], 0.0)
                bsAi = pstat.tile([128, LTILE], F32, tag="st", name="bsAi")
                bsBi = pstat.tile([128, LTILE], F32, tag="st", name="bsBi")
                bsA = pstat.tile([128, LTILE], F32, tag="st", name="bsA")
                bsB = pstat.tile([128, LTILE], F32, tag="st", name="bsB")
                pv = psPV.tile(
                    [128, NCHUNK, LTILE], F32, tag="pvall", name="pv"
                )
                # consume gathered K/V in collective-arrival order:
                # slot rk*4+lt, ordered by lt (the collective issue order)
                SLOTS = [0, 4, 1, 5, 2, 6, 3, 7]
                for jj in range(NPAIR):
                    sT = psS.tile([128, 2, LTILE], F32, tag="ps")
                    for half in range(2):
                        j = 2 * jj + half
                        slt, mc = SLOTS[j // NCHUNK], j % NCHUNK
                        for oc in range(NCHUNK):
                            nc.tensor.matmul(
                                sT[:, half, :],
                                kv_gath[:, slt, oc, mc * 128:(mc + 1) * 128],
                                qT[:, oc, :],
                                start=(oc == 0),
                                stop=(oc == NCHUNK - 1),
                            )
                    pT = pp.tile([128, 2, LTILE], BF16, tag="ppb", name="pT")
                    nc.scalar.activation(
                        pT[:], sT[:], AF.Exp, scale=inv_sqrt_c
                    )
                    acc = sumsA if jj < NPAIR // 2 else sumsB
                    nc.vector.tensor_tensor(
                        acc.rearrange("p (a b) -> p a b", a=2), acc.rearrange("p (a b) -> p a b", a=2), pT[:], ALU.add
                    )
                    for half in range(2):
                        j = 2 * jj + half
                        slt, mc = SLOTS[j // NCHUNK], j % NCHUNK
                        for cc in range(NCHUNK):
                            nc.tensor.matmul(
                                pv[:, cc, :],
                                kv_gath[:, slt, 4 + mc, cc * 128:(cc + 1) * 128],
                                pT[:, half, :],
                                start=(j == 0),
                                stop=(j == MCHUNK - 1),
                            )
                    if jj == NPAIR // 2 - 1:
                        nc.vector.tensor_tensor(
                            bsAi, sumsA[:, 0:LTILE], sumsA[:, LTILE:2 * LTILE],
                            ALU.add,
                        )
                        nc.gpsimd.partition_all_reduce(
                            bsA[:], bsAi[:], 128, bass_isa.ReduceOp.add
                        )
                nc.vector.tensor_tensor(
                    bsBi, sumsB[:, 0:LTILE], sumsB[:, LTILE:2 * LTILE], ALU.add
                )
                nc.gpsimd.partition_all_reduce(
                    bsB[:], bsBi[:], 128, bass_isa.ReduceOp.add
                )
                bs = pstat.tile([128, LTILE], F32, tag="st", name="bs")
                nc.vector.tensor_tensor(bs, bsA, bsB, ALU.add)
                rs = pstat.tile([128, LTILE], F32, tag="st", name="rs")
                nc.vector.reciprocal_approx_fast(out=rs[:], in_=bs[:])
                # unnormalized O^T -> bf16 (normalization deferred past out-proj)
                on = pon.tile([128, NCHUNK, LTILE], BF16, tag="on", name="on", bufs=2)
                nc.scalar.copy(on[:], pv[:])
                # out-projection (reuses the PV PSUM banks); then normalize,
                # SELU, residual
                po = psPV.tile(
                    [128, NCHUNK, LTILE], F32, tag="pvall", name="po"
                )
                for oc in range(NCHUNK):
                    for cc in range(NCHUNK):
                        nc.tensor.matmul(
                            po[:, oc, :],
                            wp_s[:, cc, oc * 128:(oc + 1) * 128],
                            on[:, cc, :],
                            start=(cc == 0),
                            stop=(cc == NCHUNK - 1),
                        )
                if True:
                    for ohc in range(NCHUNK):
                        z = pp.tile([128, LTILE], F32, tag="pp", name="z")
                        nc.vector.tensor_tensor(z, po[:, ohc, :], rs, ALU.mult)
                        e = pp.tile([128, LTILE], F32, tag="pp", name="e")
                        nc.scalar.activation(e, z[:], AF.Exp)
                        # e <- LA * min(e, 1)
                        nc.vector.tensor_scalar(
                            e, e, 1.0, LA, op0=ALU.min, op1=ALU.mult
                        )
                        b2 = pp.tile([128, LTILE], F32, tag="pp", name="b2")
                        # b2 <- SCALE * max(z, 0)
                        nc.vector.tensor_scalar(
                            b2, z[:], 0.0, SELU_SCALE, op0=ALU.max, op1=ALU.mult
                        )
                        nc.vector.tensor_tensor(e, e, b2, ALU.add)
                        nc.vector.tensor_scalar(e, e, LA, None, op0=ALU.subtract)
                        ot = pp.tile([128, LTILE], F32, tag="ot", name="ot", bufs=3)
                        nc.vector.tensor_tensor(ot, e, xrs[ohc], ALU.add)
                        nc.sync.dma_start(
                            out=out_d[ohc * 128:(ohc + 1) * 128, ls:ls + LTILE],
                            in_=ot[:],
                        )

    nc.compile()
    return nc


_CACHED_NC = None


def _get_nc():
    global _CACHED_NC
    if _CACHED_NC is None:
        _CACHED_NC = build_nc()
    return _CACHED_NC


def make_in_maps(x, ln_w, ln_b, wq, bq, wk, bk, wv, bv, wp, bp):
    x = np.ascontiguousarray(np.asarray(x, np.float32))
    ln_w = np.asarray(ln_w, np.float32)
    ln_b = np.asarray(ln_b, np.float32)

    def eff(w, b):
        w = np.asarray(w, np.float32)
        b = np.asarray(b, np.float32)
        w_eff = w * ln_w[None, :]
        b_eff = w @ ln_b + b
        return np.ascontiguousarray(w_eff.T), b_eff

    wqT, bq_e = eff(wq, bq)
    wkT, bk_e = eff(wk, bk)
    wvT, bv_e = eff(wv, bv)
    wpT = np.ascontiguousarray(np.asarray(wp, np.float32).T)
    assert not np.any(bv_e), "nonzero v bias not supported by the graph"
    assert not np.any(np.asarray(bp, np.float32)), "nonzero p bias not supported"
    bqk = np.ascontiguousarray(np.stack([bq_e, bk_e]).reshape(2, NCHUNK, 128))

    in_maps = []
    for i in range(8):
        b, h = i // 2, i % 2
        if h == 0:
            xs = x[b]
        else:
            xs = np.ascontiguousarray(
                np.concatenate([x[b][:, HALF:], x[b][:, :HALF]], axis=1)
            )
        in_maps.append(
            {
                "x": xs,
                "wqT": wqT,
                "wkT": wkT,
                "wvT": wvT,
                "wpT": wpT,
                "bqk": bqk,
            }
        )
    return in_maps


def assemble(results):
    out = np.empty((B, C, L), np.float32)
    for i in range(8):
        b, h = i // 2, i % 2
        out[b][:, h * HALF:(h + 1) * HALF] = results[i]["out"]
    return out


def kernel(**inputs):
    nc = _get_nc()
    in_maps = make_in_maps(**inputs)
    res = run_bass_kernel_spmd(nc, in_maps, core_ids=list(range(8)))
    return assemble(res.results)


if __name__ == "__main__":
    build_nc()
    print("built + compiled OK")
